# revision 22
# baseline (speedup 1.0000x reference)
"""Trainium2 Bass kernel for nn_CenterContrastiveLoss.

Problem: loss = label-smoothed CE over [pos, top50_negs] of f @ centers.T
  f: [2048, 256] f32, centers: [65536, 256] f32, label: [2048] int.

Strategy (8 NeuronCores, tensor-parallel over C=65536):
  - Each core takes an 8192-column shard of centers and computes
    S = f @ shard.T in bf16 (PSUM f32 accumulate), streamed through PSUM
    in [128 rows x 2048 cols] supertiles.
  - Per row-tile, supertiles 0-2 are evicted by ScalarE as exp(S-60) bf16
    (monotone, log-precision ~0.004) and reduced by VectorE with a
    pairwise-max tree at the 2x bf16 rate down to 128 bucket-maxes.
    Supertile 3 is evicted by VectorE directly as a fused PSUM->f16
    grouped max-reduce (16 raw-domain buckets), offloading ScalarE,
    which is otherwise the pipeline bottleneck.
  - Host merges 8 x (128 exp + 16 raw) bucket-maxes per row: exact-enough
    top-50 values (S1), the LSE (tail below candidates is ~1e-9), and the
    positive handled via value-matching + exact f32 recompute.
"""

import numpy as np
import ml_dtypes

B, C, D = 2048, 65536, 256
NCORES = 8
CSH = C // NCORES          # 8192 columns per core
RT = B // 128              # 16 row tiles
ST = 4                     # supertiles per row tile (2048 cols each)
STW = CSH // ST            # 2048
NEXP = 512                 # exp-domain bucket maxes per row per core
NRAW = 32                  # raw-domain bucket maxes (supertile 0)
SHIFT = 60.0

_prog = None


def _build_program():
    import concourse.mybir as mybir
    from concourse import bacc
    from concourse.tile import TileContext
    from contextlib import ExitStack

    bf16 = mybir.dt.bfloat16
    f16 = mybir.dt.float16
    f32 = mybir.dt.float32

    # Bacc (not raw Bass): its compile() legalizes sync waits (TRN2 allows
    # only 1 wait/instruction), inserts ACT table loads, and lowers
    # extended-ISA instructions.
    nc = bacc.Bacc("TRN2")
    fT_d = nc.declare_dram_parameter("fT", [2, 128, B], bf16, isOutput=False)
    cT_d = nc.declare_dram_parameter("cT", [2, 128, CSH], bf16, isOutput=False)
    exp_d = nc.declare_dram_parameter("out_exp", [RT, 128, NEXP], bf16, isOutput=True)
    raw_d = nc.declare_dram_parameter("out_raw", [RT, 128, NRAW], f16, isOutput=True)

    with TileContext(nc) as tc, ExitStack() as ctx:
        const = ctx.enter_context(tc.tile_pool(name="const", bufs=1))
        psum = ctx.enter_context(tc.tile_pool(name="psum", bufs=2, space="PSUM"))
        scr = ctx.enter_context(tc.tile_pool(name="scr", bufs=3))
        outp = ctx.enter_context(tc.tile_pool(name="outp", bufs=2))

        fT_t = [const.tile([128, B], bf16, tag=f"fT{k}", name=f"fT{k}")
                for k in range(2)]
        # one tile per (k, supertile-column) so DMA deps are precise and the
        # first matmuls start as soon as their own chunk lands
        cT_t = [[const.tile([128, STW], bf16, tag=f"cT{k}_{q}",
                            name=f"cT{k}_{q}") for q in range(4)]
                for k in range(2)]
        bias_t = const.tile([128, 1], f32, tag="bias", name="bias")
        nc.vector.memset(bias_t[:], -SHIFT)
        nc.sync.dma_start(out=fT_t[0][:], in_=fT_d[0])
        nc.sync.dma_start(out=cT_t[0][0][:], in_=cT_d[0, :, 0:STW])
        nc.sync.dma_start(out=fT_t[1][:], in_=fT_d[1])
        nc.sync.dma_start(out=cT_t[1][0][:], in_=cT_d[1, :, 0:STW])
        for q in range(1, 4):
            for k in range(2):
                nc.sync.dma_start(out=cT_t[k][q][:],
                                  in_=cT_d[k, :, q * STW:(q + 1) * STW])

        for rt in range(RT):
            et = scr.tile([128, 3 * STW], bf16, tag="et")
            raw_t = outp.tile([128, NRAW], f16, tag="raw")
            tr = scr.tile([128, STW], bf16, tag="tr")
            # s=0 is the raw quarter (VectorE eviction) so DVE's heavy op
            # overlaps ScalarE's exps of s=1..3 instead of colliding with
            # the tree at the row-tile boundary
            for s in range(ST):
                pt = psum.tile([128, STW], f32, tag="pt")
                for k in range(2):
                    lhsT = fT_t[k][:, rt * 128:(rt + 1) * 128]
                    for c in range(ST):
                        nc.tensor.matmul(
                            pt[:, c * 512:(c + 1) * 512],
                            lhsT,
                            cT_t[k][s][:, c * 512:(c + 1) * 512],
                            start=(k == 0),
                            stop=(k == 1),
                        )
                if s == 0:
                    # fused eviction+reduce of the raw quarter on VectorE
                    nc.vector.tensor_reduce(
                        out=raw_t[:],
                        in_=pt[:].rearrange("p (g e) -> p g e", e=STW // NRAW),
                        axis=mybir.AxisListType.X,
                        op=mybir.AluOpType.max,
                    )
                else:
                    nc.scalar.activation(
                        out=et[:, (s - 1) * STW:s * STW],
                        in_=pt[:],
                        func=mybir.ActivationFunctionType.Exp,
                        bias=bias_t[:],
                        scale=1.0,
                    )
                # max-funnel 6144 -> 512 exp buckets, interleaved with matmuls
                if s == 3:
                    nc.vector.tensor_max(tr[:], et[:, 0:STW], et[:, STW:2 * STW])
            nc.vector.tensor_max(tr[:], tr[:], et[:, 2 * STW:3 * STW])
            nc.vector.tensor_max(tr[:, 0:1024], tr[:, 0:1024], tr[:, 1024:2048])
            nc.vector.tensor_max(tr[:, 0:NEXP], tr[:, 0:NEXP],
                                 tr[:, NEXP:2 * NEXP])
            nc.sync.dma_start(out=exp_d[rt], in_=tr[:, 0:NEXP])
            nc.sync.dma_start(out=raw_d[rt], in_=raw_t[:])

    nc.finalize()  # Bacc: compile() legalization + freeze
    return nc


def _get_program():
    global _prog
    if _prog is None:
        _prog = _build_program()
    return _prog


def run_device(in_maps, trace=False, **kw):
    from concourse.bass_utils import run_bass_kernel_spmd

    nc = _get_program()
    return run_bass_kernel_spmd(nc, in_maps, core_ids=list(range(NCORES)),
                                trace=trace, **kw)


def make_in_maps(f, centers, label):
    bf16 = ml_dtypes.bfloat16
    fb = f.astype(bf16)
    cb = centers.astype(bf16)
    fT = np.ascontiguousarray(fb.T).reshape(2, 128, B)
    in_maps = []
    for core in range(NCORES):
        cT = np.ascontiguousarray(
            cb[core * CSH:(core + 1) * CSH].T).reshape(2, 128, CSH)
        in_maps.append({"fT": fT, "cT": cT})
    return in_maps


def postprocess(results, f, centers, label):
    """Combine per-core bucket maxes into the scalar loss (float64 host)."""
    rows = np.arange(B)
    exp_c = np.concatenate(
        [np.asarray(r["out_exp"], dtype=np.float64).reshape(B, NEXP)
         for r in results], axis=1)                  # [B, 1024] exp domain
    raw_c = np.concatenate(
        [np.asarray(r["out_raw"], dtype=np.float64).reshape(B, NRAW)
         for r in results], axis=1)                  # [B, 128] raw domain

    bf16 = ml_dtypes.bfloat16
    fb = f.astype(bf16).astype(np.float32)
    pcb = centers[label].astype(bf16).astype(np.float32)
    # emulate the device's bf16 matmul value of the positive (within ~1e-4)
    pd = np.sum(fb * pcb, axis=1, dtype=np.float32).astype(np.float64)
    pos_f32 = np.einsum("ij,ij->i", f.astype(np.float64),
                        centers[label].astype(np.float64))

    cand_raw = np.concatenate(
        [SHIFT + np.log(np.maximum(exp_c, 1e-300)), raw_c], axis=1)
    win = np.concatenate([np.full(exp_c.shape[1], 0.02),
                          np.full(raw_c.shape[1], 0.12)])
    diff = np.abs(cand_raw - pd[:, None])
    diffm = np.where(diff < win[None, :], diff, np.inf)
    j = np.argmin(diffm, axis=1)
    hit = np.isfinite(diffm[rows, j])
    cand_raw[rows[hit], j[hit]] = -np.inf           # drop the positive

    top50 = -np.sort(-cand_raw, axis=1)[:, :50]
    S1 = top50.sum(axis=1)
    # LSE over {pos} ∪ negs: tail below candidates contributes ~1e-9
    se_neg = np.exp(cand_raw - SHIFT,
                    where=np.isfinite(cand_raw),
                    out=np.zeros_like(cand_raw)).sum(axis=1)
    lse = SHIFT + np.log(se_neg + np.exp(pos_f32 - SHIFT))
    loss = (0.9102 * lse - 0.9002 * pos_f32 - 0.0002 * S1).mean()
    return np.float32(loss)


def kernel(f, centers, label):
    f = np.asarray(f, dtype=np.float32)
    centers = np.asarray(centers, dtype=np.float32)
    label = np.asarray(label).astype(np.int64)
    in_maps = make_in_maps(f, centers, label)
    res = run_device(in_maps)
    return postprocess(res.results, f, centers, label)


# revision 24
# speedup vs baseline: 1.1716x; 1.1716x over previous
"""Trainium2 Bass kernel for nn_CenterContrastiveLoss.

Problem: loss = label-smoothed CE over [pos, top50_negs] of f @ centers.T
  f: [2048, 256] f32, centers: [65536, 256] f32, label: [2048] int.

Strategy (8 NeuronCores, tensor-parallel over C=65536):
  - Each core takes an 8192-column shard of centers and computes
    S = f @ shard.T in bf16 (PSUM f32 accumulate), streamed through PSUM
    in [128 rows x 2048 cols] supertiles.
  - Per row-tile, supertiles 0-2 are evicted by ScalarE as exp(S-60) bf16
    (monotone, log-precision ~0.004) and reduced by VectorE with a
    pairwise-max tree at the 2x bf16 rate down to 128 bucket-maxes.
    Supertile 3 is evicted by VectorE directly as a fused PSUM->f16
    grouped max-reduce (16 raw-domain buckets), offloading ScalarE,
    which is otherwise the pipeline bottleneck.
  - Host merges 8 x (128 exp + 16 raw) bucket-maxes per row: exact-enough
    top-50 values (S1), the LSE (tail below candidates is ~1e-9), and the
    positive handled via value-matching + exact f32 recompute.
"""

import numpy as np
import ml_dtypes

B, C, D = 2048, 65536, 256
NCORES = 8
CSH = C // NCORES          # 8192 columns per core
RT = B // 128              # 16 row tiles
ST = 4                     # supertiles per row tile (2048 cols each)
STW = CSH // ST            # 2048
NEXP = 512                 # exp-domain bucket maxes per row per core
NRAW = 32                  # raw-domain bucket maxes (supertile 0)
SHIFT = 60.0

_prog = None


def _build_program():
    import concourse.mybir as mybir
    from concourse import bacc
    from concourse.tile import TileContext
    from contextlib import ExitStack

    bf16 = mybir.dt.bfloat16
    f16 = mybir.dt.float16
    f32 = mybir.dt.float32

    # Bacc (not raw Bass): its compile() legalizes sync waits (TRN2 allows
    # only 1 wait/instruction), inserts ACT table loads, and lowers
    # extended-ISA instructions.
    nc = bacc.Bacc("TRN2")
    fT_d = nc.declare_dram_parameter("fT", [2, 128, B], bf16, isOutput=False)
    cT_d = nc.declare_dram_parameter("cT", [2, 128, CSH], bf16, isOutput=False)
    exp_d = nc.declare_dram_parameter("out_exp", [RT, 128, NEXP], bf16, isOutput=True)
    raw_d = nc.declare_dram_parameter("out_raw", [RT, 128, NRAW], f16, isOutput=True)

    with TileContext(nc) as tc, ExitStack() as ctx:
        const = ctx.enter_context(tc.tile_pool(name="const", bufs=1))
        psum = ctx.enter_context(tc.tile_pool(name="psum", bufs=4, space="PSUM"))
        scr = ctx.enter_context(tc.tile_pool(name="scr", bufs=3))
        outp = ctx.enter_context(tc.tile_pool(name="outp", bufs=2))

        fT_t = [const.tile([128, B], bf16, tag=f"fT{k}", name=f"fT{k}")
                for k in range(2)]
        # one tile per (k, supertile-column) so DMA deps are precise and the
        # first matmuls start as soon as their own chunk lands
        cT_t = [[const.tile([128, STW], bf16, tag=f"cT{k}_{q}",
                            name=f"cT{k}_{q}") for q in range(4)]
                for k in range(2)]
        bias_t = const.tile([128, 1], f32, tag="bias", name="bias")
        nc.vector.memset(bias_t[:], -SHIFT)
        nc.sync.dma_start(out=fT_t[0][:], in_=fT_d[0])
        nc.sync.dma_start(out=cT_t[0][0][:], in_=cT_d[0, :, 0:STW])
        nc.sync.dma_start(out=fT_t[1][:], in_=fT_d[1])
        nc.sync.dma_start(out=cT_t[1][0][:], in_=cT_d[1, :, 0:STW])
        for q in range(1, 4):
            for k in range(2):
                nc.sync.dma_start(out=cT_t[k][q][:],
                                  in_=cT_d[k, :, q * STW:(q + 1) * STW])

        SW = 1024              # supertile width: [128,1024] = 2 PSUM banks, 4 slots
        for rt in range(RT):
            et = scr.tile([128, 3 * STW], bf16, tag="et")
            raw_t = outp.tile([128, NRAW], f16, tag="raw")
            tr = scr.tile([128, STW], bf16, tag="tr")
            # g=0,1 are the raw quarter (VectorE eviction) so DVE's heavy ops
            # overlap ScalarE's exps of g=2..7 instead of colliding with
            # the funnel at the row-tile boundary
            for g in range(8):
                q, h = g // 2, g % 2
                pt = psum.tile([128, SW], f32, tag="pt")
                for k in range(2):
                    lhsT = fT_t[k][:, rt * 128:(rt + 1) * 128]
                    for c in range(2):
                        nc.tensor.matmul(
                            pt[:, c * 512:(c + 1) * 512],
                            lhsT,
                            cT_t[k][q][:, h * SW + c * 512:h * SW + (c + 1) * 512],
                            start=(k == 0),
                            stop=(k == 1),
                        )
                if g < 2:
                    # fused eviction+reduce of the raw quarter on VectorE
                    nc.vector.tensor_reduce(
                        out=raw_t[:, g * (NRAW // 2):(g + 1) * (NRAW // 2)],
                        in_=pt[:].rearrange("p (g e) -> p g e",
                                            e=SW // (NRAW // 2)),
                        axis=mybir.AxisListType.X,
                        op=mybir.AluOpType.max,
                    )
                else:
                    nc.scalar.activation(
                        out=et[:, (g - 2) * SW:(g - 1) * SW],
                        in_=pt[:],
                        func=mybir.ActivationFunctionType.Exp,
                        bias=bias_t[:],
                        scale=1.0,
                    )
                # max-funnel 6144 -> 512 exp buckets, interleaved
                if g == 6:
                    nc.vector.tensor_max(tr[:], et[:, 0:STW], et[:, STW:2 * STW])
            nc.vector.tensor_max(tr[:], tr[:], et[:, 2 * STW:3 * STW])
            nc.vector.tensor_max(tr[:, 0:1024], tr[:, 0:1024], tr[:, 1024:2048])
            nc.vector.tensor_max(tr[:, 0:NEXP], tr[:, 0:NEXP],
                                 tr[:, NEXP:2 * NEXP])
            nc.sync.dma_start(out=exp_d[rt], in_=tr[:, 0:NEXP])
            nc.sync.dma_start(out=raw_d[rt], in_=raw_t[:])

    nc.finalize()  # Bacc: compile() legalization + freeze
    return nc


def _get_program():
    global _prog
    if _prog is None:
        _prog = _build_program()
    return _prog


def run_device(in_maps, trace=False, **kw):
    from concourse.bass_utils import run_bass_kernel_spmd

    nc = _get_program()
    return run_bass_kernel_spmd(nc, in_maps, core_ids=list(range(NCORES)),
                                trace=trace, **kw)


def make_in_maps(f, centers, label):
    bf16 = ml_dtypes.bfloat16
    fb = f.astype(bf16)
    cb = centers.astype(bf16)
    fT = np.ascontiguousarray(fb.T).reshape(2, 128, B)
    in_maps = []
    for core in range(NCORES):
        cT = np.ascontiguousarray(
            cb[core * CSH:(core + 1) * CSH].T).reshape(2, 128, CSH)
        in_maps.append({"fT": fT, "cT": cT})
    return in_maps


def postprocess(results, f, centers, label):
    """Combine per-core bucket maxes into the scalar loss (float64 host)."""
    rows = np.arange(B)
    exp_c = np.concatenate(
        [np.asarray(r["out_exp"], dtype=np.float64).reshape(B, NEXP)
         for r in results], axis=1)                  # [B, 1024] exp domain
    raw_c = np.concatenate(
        [np.asarray(r["out_raw"], dtype=np.float64).reshape(B, NRAW)
         for r in results], axis=1)                  # [B, 128] raw domain

    bf16 = ml_dtypes.bfloat16
    fb = f.astype(bf16).astype(np.float32)
    pcb = centers[label].astype(bf16).astype(np.float32)
    # emulate the device's bf16 matmul value of the positive (within ~1e-4)
    pd = np.sum(fb * pcb, axis=1, dtype=np.float32).astype(np.float64)
    pos_f32 = np.einsum("ij,ij->i", f.astype(np.float64),
                        centers[label].astype(np.float64))

    cand_raw = np.concatenate(
        [SHIFT + np.log(np.maximum(exp_c, 1e-300)), raw_c], axis=1)
    win = np.concatenate([np.full(exp_c.shape[1], 0.02),
                          np.full(raw_c.shape[1], 0.12)])
    diff = np.abs(cand_raw - pd[:, None])
    diffm = np.where(diff < win[None, :], diff, np.inf)
    j = np.argmin(diffm, axis=1)
    hit = np.isfinite(diffm[rows, j])
    cand_raw[rows[hit], j[hit]] = -np.inf           # drop the positive

    top50 = -np.sort(-cand_raw, axis=1)[:, :50]
    S1 = top50.sum(axis=1)
    # LSE over {pos} ∪ negs: tail below candidates contributes ~1e-9
    se_neg = np.exp(cand_raw - SHIFT,
                    where=np.isfinite(cand_raw),
                    out=np.zeros_like(cand_raw)).sum(axis=1)
    lse = SHIFT + np.log(se_neg + np.exp(pos_f32 - SHIFT))
    loss = (0.9102 * lse - 0.9002 * pos_f32 - 0.0002 * S1).mean()
    return np.float32(loss)


def kernel(f, centers, label):
    f = np.asarray(f, dtype=np.float32)
    centers = np.asarray(centers, dtype=np.float32)
    label = np.asarray(label).astype(np.int64)
    in_maps = make_in_maps(f, centers, label)
    res = run_device(in_maps)
    return postprocess(res.results, f, centers, label)
